# revision 1
# baseline (speedup 1.0000x reference)
"""Causal MHA kernel for 8 TRN2 NeuronCores.

Problem: x[4,2048,1024], 16 heads, hd=64, causal softmax attention, f32.

Sharding: core c handles batch c%4 and head-half c//4 (8 heads).
Each core computes its 8 heads' attention plus the row-slice of the
output projection; the host sums the two partials per batch (the
all-reduce of the row-parallel W_o split) and adds b_o.

Device-side layout: everything transposed. Host ships x[b].T so the
contraction dim (D) lands on SBUF partitions with fast DMA. Projections
produce Q^T/K^T/V^T [64*heads, t]; scores = K_tile^T . Q chunk in PSUM
(S^T layout: keys on partitions, queries on free dim); exp on ScalarE;
causal mask via affine_select (fill=0) on diagonal tiles only; ctx^T
accumulated with V_ext stationary tiles that carry a ones-column so
PSUM row 64 collects the softmax denominators.
"""

import os

import numpy as np

B, S, D, H, HD = 4, 2048, 1024, 16, 64
HL = 8            # heads per core
F = HL * HD       # 512 local head features
P = 128
CH = 512          # free-dim chunk for matmuls
NKT = D // P      # 8 contraction tiles for projections
NMT = F // P      # 4 head-pair tiles
NCH = S // CH     # 4 token chunks
NKA = S // P      # 16 attention key tiles

_NC_CACHE = {}


def _build_nc(reps=1):
    from contextlib import ExitStack

    import concourse.bass as bass
    import concourse.tile as tile
    from concourse import bacc, mybir
    from concourse.masks import make_identity

    f32 = mybir.dt.float32
    f32r = mybir.dt.float32r
    AF = mybir.ActivationFunctionType
    ALU = mybir.AluOpType

    def r(ap):
        return ap.bitcast(f32r)

    nc = bacc.Bacc("TRN2", target_bir_lowering=False)
    xt_d = nc.declare_dram_parameter("xt", [D, S], f32, isOutput=False)
    wq_d = nc.declare_dram_parameter("wq", [D, F], f32, isOutput=False)
    wk_d = nc.declare_dram_parameter("wk", [D, F], f32, isOutput=False)
    wv_d = nc.declare_dram_parameter("wv", [D, F], f32, isOutput=False)
    wo_d = nc.declare_dram_parameter("wo", [F, D], f32, isOutput=False)
    out_d = nc.declare_dram_parameter("out", [S, D], f32, isOutput=True)
    w_by_name = {"q": wq_d, "k": wk_d, "v": wv_d}

    with tile.TileContext(nc) as tc, ExitStack() as ctx:
        const_pool = ctx.enter_context(tc.tile_pool(name="const", bufs=1))
        qt_pool = ctx.enter_context(tc.tile_pool(name="qt", bufs=1))
        ve_pool = ctx.enter_context(tc.tile_pool(name="ve", bufs=1))
        wo_pool = ctx.enter_context(tc.tile_pool(name="wo", bufs=1))

        ident = const_pool.tile([P, P], f32)
        make_identity(nc, ident[:])
        onesf = const_pool.tile([P, 1], f32)
        nc.vector.memset(onesf[:], 1.0)
        ones_row = const_pool.tile([1, P], f32r)
        nc.vector.tensor_copy(ones_row[:], onesf[0:1, 0:1].broadcast_to([1, P]))

        QT = [qt_pool.tile([P, S], f32r, name=f"qt{m}", tag=f"qt{m}")
              for m in range(NMT)]
        KT = [qt_pool.tile([P, S], f32r, name=f"kt{m}", tag=f"kt{m}")
              for m in range(NMT)]
        # V_ext: per (head, key-tile) a [128, 65] stationary block; col 64
        # stays 1.0 (single memset; projection copies only touch cols 0..63).
        VE = ve_pool.tile([P, HL * NKA * 65], f32r)
        nc.vector.tensor_copy(
            VE[:].rearrange("p (b c) -> p b c", c=65)[:, :, 64:65],
            onesf[:].broadcast_to([P, HL * NKA, 1]),
        )

        masks = const_pool.tile([P, 4 * CH], f32)
        nc.vector.memset(masks[:], 1.0)
        for j in range(4):
            nc.gpsimd.affine_select(
                out=masks[:, j * CH : (j + 1) * CH],
                in_=masks[:, j * CH : (j + 1) * CH],
                compare_op=ALU.is_ge,
                fill=0.0,
                base=-j * P,
                pattern=[[1, CH]],
                channel_multiplier=-1,
            )

        WO = wo_pool.tile([P, NMT * D], f32r)
        nc.gpsimd.dma_start(
            WO[:].rearrange("p (f n) -> p f n", f=NMT),
            r(wo_d[:]).rearrange("(f p) n -> p f n", p=P),
        )

        for _rep in range(reps):
            # ---- Phase 1: projections -------------------------------------
            with tc.tile_pool(name="xt", bufs=2) as xt_pool, \
                 tc.tile_pool(name="ws", bufs=2) as ws_pool, \
                 tc.tile_pool(name="vstage", bufs=2) as vs_pool, \
                 tc.tile_pool(name="pp", bufs=2, space="PSUM") as pp_pool, \
                 tc.tile_pool(name="pt", bufs=2, space="PSUM") as pt_pool:
                for tch in range(NCH):
                    xt_t = xt_pool.tile([P, NKT * CH], f32r)
                    nc.gpsimd.dma_start(
                        xt_t[:].rearrange("p (k t) -> p k t", k=NKT),
                        r(xt_d[:]).rearrange("(k p) t -> p k t", p=P)[
                            :, :, tch * CH : (tch + 1) * CH
                        ],
                    )
                    for wname in ("v", "k", "q"):
                        wd = w_by_name[wname]
                        ws = ws_pool.tile([P, NKT * F], f32r)
                        nc.gpsimd.dma_start(
                            ws[:].rearrange("p (k f) -> p k f", k=NKT),
                            r(wd[:]).rearrange("(k p) f -> p k f", p=P),
                        )
                        for mt in range(NMT):
                            pp = pp_pool.tile([P, CH], f32)
                            for kt in range(NKT):
                                nc.tensor.matmul(
                                    pp[:],
                                    ws[:, kt * F + mt * P : kt * F + (mt + 1) * P],
                                    xt_t[:, kt * CH : (kt + 1) * CH],
                                    start=(kt == 0),
                                    stop=(kt == NKT - 1),
                                )
                            if wname == "q":
                                nc.vector.tensor_copy(
                                    QT[mt][:, tch * CH : (tch + 1) * CH], pp[:]
                                )
                            elif wname == "k":
                                nc.vector.tensor_copy(
                                    KT[mt][:, tch * CH : (tch + 1) * CH], pp[:]
                                )
                            else:
                                vs = vs_pool.tile([P, CH], f32)
                                nc.scalar.copy(vs[:], pp[:])
                                for j in range(CH // P):
                                    ka = tch * (CH // P) + j
                                    ptp = pt_pool.tile([P, P], f32)
                                    nc.tensor.transpose(
                                        ptp[:], vs[:, j * P : (j + 1) * P], ident[:]
                                    )
                                    for hh in range(2):
                                        h = 2 * mt + hh
                                        col = (h * NKA + ka) * 65
                                        nc.scalar.copy(
                                            VE[:, col : col + HD],
                                            ptp[:, hh * HD : (hh + 1) * HD],
                                        )

            # ---- Phase 2+3: attention fused with output projection --------
            with tc.tile_pool(name="ptile", bufs=3) as ptile_pool, \
                 tc.tile_pool(name="ctc", bufs=2) as ctc_pool, \
                 tc.tile_pool(name="rec", bufs=2) as rec_pool, \
                 tc.tile_pool(name="bsb", bufs=2) as bsb_pool, \
                 tc.tile_pool(name="osb", bufs=2) as osb_pool, \
                 tc.tile_pool(name="ps_s", bufs=2, space="PSUM") as ps_s_pool, \
                 tc.tile_pool(name="ps_c", bufs=2, space="PSUM") as ps_c_pool, \
                 tc.tile_pool(name="ps_b", bufs=2, space="PSUM") as ps_b_pool:
                for qc in range(NCH):
                    ctc = [ctc_pool.tile([P, CH], f32r, name=f"ctc{m}", tag=f"ctc{m}")
                           for m in range(NMT)]
                    for h in range(HL):
                        mt = h // 2
                        hrow = (h % 2) * HD
                        nka_q = 4 * qc + 4  # causal: key tiles 0..nka_q-1
                        pc = ps_c_pool.tile([HD + 1, CH], f32, tag="pc")
                        for kt2 in range(0, nka_q, 2):
                            ps2 = ps_s_pool.tile([P, 2 * CH], f32)
                            pt2 = ptile_pool.tile([P, 2 * CH], f32r)
                            for u in range(2):
                                kt = kt2 + u
                                nc.tensor.matmul(
                                    ps2[:, u * CH : (u + 1) * CH],
                                    KT[mt][hrow : hrow + HD,
                                           kt * P : (kt + 1) * P],
                                    QT[mt][hrow : hrow + HD,
                                           qc * CH : (qc + 1) * CH],
                                    start=True,
                                    stop=True,
                                )
                            nc.scalar.activation(
                                pt2[:], ps2[:], AF.Exp, scale=0.125
                            )
                            for u in range(2):
                                kt = kt2 + u
                                if kt >= 4 * qc:  # diagonal tile: mask
                                    j = kt - 4 * qc
                                    nc.vector.tensor_mul(
                                        pt2[:, u * CH : (u + 1) * CH],
                                        pt2[:, u * CH : (u + 1) * CH],
                                        masks[:, j * CH : (j + 1) * CH],
                                    )
                                col = (h * NKA + kt) * 65
                                nc.tensor.matmul(
                                    pc[:],
                                    VE[:, col : col + HD + 1],
                                    pt2[:, u * CH : (u + 1) * CH],
                                    start=(kt == 0),
                                    stop=(kt == nka_q - 1),
                                )
                        rec = rec_pool.tile([1, CH], f32r)
                        with nc.allow_low_precision(
                            reason="1/l rounded to fp32r for PE broadcast"
                        ):
                            nc.vector.reciprocal(rec[:], pc[HD : HD + 1, :])
                        pb = ps_b_pool.tile([P, CH], f32)
                        nc.tensor.matmul(
                            pb[:], ones_row[:], rec[:], start=True, stop=True
                        )
                        bsb = bsb_pool.tile([P, CH], f32)
                        nc.vector.tensor_copy(bsb[:], pb[:])
                        nc.vector.tensor_mul(
                            ctc[mt][hrow : hrow + HD, :],
                            pc[0:HD, :],
                            bsb[0:HD, :],
                        )
                    # output projection for this token chunk
                    for tt4 in range(CH // P):
                        osb = osb_pool.tile([P, D], f32)
                        for ncol in range(D // CH):
                            po = ps_c_pool.tile([P, CH], f32, tag="pc")
                            for ft in range(NMT):
                                nc.tensor.matmul(
                                    po[:],
                                    ctc[ft][:, tt4 * P : (tt4 + 1) * P],
                                    WO[:, ft * D + ncol * CH
                                       : ft * D + (ncol + 1) * CH],
                                    start=(ft == 0),
                                    stop=(ft == NMT - 1),
                                )
                            if ncol % 2 == 0:
                                nc.vector.tensor_copy(
                                    osb[:, ncol * CH : (ncol + 1) * CH], po[:]
                                )
                            else:
                                nc.scalar.copy(
                                    osb[:, ncol * CH : (ncol + 1) * CH], po[:]
                                )
                        r0 = qc * CH + tt4 * P
                        nc.gpsimd.dma_start(out_d[r0 : r0 + P, :], osb[:])

    nc.compile()
    return nc


def _get_nc(reps=1):
    key = f"nc{reps}"
    if key not in _NC_CACHE:
        _NC_CACHE[key] = _build_nc(reps)
    return _NC_CACHE[key]


def _make_in_maps(inputs):
    x = np.asarray(inputs["x"], dtype=np.float32)
    W_q = np.asarray(inputs["W_q"], dtype=np.float32)
    W_k = np.asarray(inputs["W_k"], dtype=np.float32)
    W_v = np.asarray(inputs["W_v"], dtype=np.float32)
    W_o = np.asarray(inputs["W_o"], dtype=np.float32)
    in_maps = []
    for c in range(8):
        b = c % 4
        hh = c // 4
        cols = slice(hh * F, (hh + 1) * F)
        in_maps.append(
            {
                "xt": np.ascontiguousarray(x[b].T),
                "wq": np.ascontiguousarray(W_q[:, cols]),
                "wk": np.ascontiguousarray(W_k[:, cols]),
                "wv": np.ascontiguousarray(W_v[:, cols]),
                "wo": np.ascontiguousarray(W_o[cols, :]),
            }
        )
    return in_maps


def kernel(x, W_q, W_k, W_v, W_o, b_o):
    from concourse.bass_utils import run_bass_kernel_spmd

    b_o = np.asarray(b_o, dtype=np.float32)
    nc = _get_nc()
    in_maps = _make_in_maps(
        {"x": x, "W_q": W_q, "W_k": W_k, "W_v": W_v, "W_o": W_o}
    )
    res = run_bass_kernel_spmd(nc, in_maps, core_ids=list(range(8)))

    full = np.empty((B, S, D), dtype=np.float32)
    for b in range(B):
        full[b] = res.results[b]["out"] + res.results[b + 4]["out"] + b_o
    return full

